# revision 57
# baseline (speedup 1.0000x reference)
"""Trainium2 Bass kernel for a dense transformer block (RMSNorm + GQA attention
with RoPE + SwiGLU MLP), tensor-parallel over 8 NeuronCores.

Megatron-style TP=8 with sequence-parallel collectives: core c owns heads
{2c, 2c+1}, KV head c, and FF rows [c*768, (c+1)*768) of an FF dim padded
5504->6144. Each core receives only its 512-token slice of x (bf16) plus its
1/8 weight shard (int8 with absmax scales, dequantized through existing
activation-scale operands), ~10MB/core: host->device transfer is the
wall-clock bottleneck under the axon tunnel (~65MB/s aggregate), so bytes
shipped, not FLOPs, set the runtime — ~16x less than a replicated-weights
layout. Device collectives stitch the block back together:
AllGather the rmsnorm'd activations (bf16) before QKV and before the MLP,
ReduceScatter the o-proj and down-proj partial sums (f32) so each core
finishes with exactly its 512 output rows.

The kernel returns only the attn+mlp contribution (bf16); the host adds the
f32 residual x back, so neither x nor the output is rounded on the residual
path (the device x copy only feeds the two rmsnorms, where its bf16 rounding
is second-order). Matmuls run in bf16 with f32 PSUM accumulation; softmax
runs without max-subtraction (scores sigma~0.8; exp cannot overflow).
Causality is exploited: query chunk qc only visits key subtiles 0..4*(qc+1)-1
of its batch, with an on-device affine_select staircase mask on the 4
diagonal subtiles.
"""

import sys

sys.path.insert(0, "/opt/trn_rl_repo")

import numpy as np
import ml_dtypes

B, S, D = 2, 2048, 2048
H, KVH, HD = 16, 8, 128
FF = 5504
P = 128
DS = D // P          # 16 subtiles of D
T = B * S            # 4096 tokens
TN = 512             # tokens per core shard
NT = T // TN         # 8 token chunks == n cores
HC = H // 8          # 2 heads per core
FFP = 768            # padded FF rows per core (6144 total)
FC = FFP // P        # 6 FF subtiles per core
EPS = 1e-5
NCORES = 8

_prog = None


def _build():
    from contextlib import ExitStack

    import concourse.bass as bass  # noqa: F401
    import concourse.tile as tile
    from concourse import bacc, mybir
    from concourse.masks import make_identity

    f32 = mybir.dt.float32
    bf16 = mybir.dt.bfloat16
    AF = mybir.ActivationFunctionType
    OP = mybir.AluOpType
    GRP = [list(range(NCORES))]

    nc = bacc.Bacc("TRN2", target_bir_lowering=False, debug=False)

    x_sh = nc.dram_tensor("x_sh", [D, TN], bf16, kind="ExternalInput").ap()
    i8_a = mybir.dt.int8
    # Attention weights ship int8 with per-head scalar scales for q/k/v (RoPE
    # is linear, so the scale rides through it; s_q*s_k folds into the Exp
    # activation scale, s_v into the o-proj row scales) and per-row scales
    # for wo. att_s col h = s_q[h]*s_k; col HC+mc = s_wo[mc*128+p]*s_v.
    wq = nc.dram_tensor("wq_pk", [HC, P, DS, P], i8_a, kind="ExternalInput").ap()
    wk = nc.dram_tensor("wk_pk", [P, DS, P], i8_a, kind="ExternalInput").ap()
    wv = nc.dram_tensor("wv_pk", [P, DS, P], i8_a, kind="ExternalInput").ap()
    wo = nc.dram_tensor("wo_pk", [DS, P, HC, P], i8_a, kind="ExternalInput").ap()
    atts = nc.dram_tensor("att_s", [P, HC + DS], f32, kind="ExternalInput").ap()
    i8 = mybir.dt.int8
    # MLP weights ship int8 (absmax/127 per-core per-tensor); the dequant
    # scale is applied to the PSUM result via the consuming activation's
    # per-partition scale AP, so only an int8->bf16 cast is added per tile.
    wg = nc.dram_tensor("wg_pk", [FC, P, DS, P], i8, kind="ExternalInput").ap()
    wu = nc.dram_tensor("wu_pk", [FC, P, DS, P], i8, kind="ExternalInput").ap()
    wd = nc.dram_tensor("wd_pk", [DS, P, FC, P], i8, kind="ExternalInput").ap()
    # Per-output-channel dequant scales: [:, j] gate, [:, FC+j] up, [:, 2FC+mc]
    # down; channel j*128+p sits on PSUM partition p, so a [P,1] slice feeds
    # the consuming activation's per-partition scale operand exactly.
    mlps = nc.dram_tensor("mlp_s", [P, 2 * FC + DS], f32, kind="ExternalInput").ap()
    # RoPE tables: rows 64..127 duplicate rows 0..63 (freqs concatenated
    # twice), so only the lower half ships; sin's pre-signing (rows 0..63
    # negated) is applied during the on-device upcast.
    cosk = nc.dram_tensor("cos_k", [HD // 2, S], bf16, kind="ExternalInput").ap()
    sink = nc.dram_tensor("sin_k", [HD // 2, S], bf16, kind="ExternalInput").ap()
    # out_rows carries only the attn+mlp contribution (bf16); the host adds
    # the f32 residual x back, so x is never rounded on the output path.
    out_rows = nc.dram_tensor("out_rows", [TN, D], bf16, kind="ExternalOutput").ap()

    # Collective buffers (inputs Local, outputs Shared).
    hsh_d = nc.dram_tensor("hsh_d", [D, TN], bf16).ap()
    hcat_d = nc.dram_tensor("hcat_d", [NT, D, TN], bf16, addr_space="Shared").ap()
    ypart_d = nc.dram_tensor("ypart_d", [NT, D, TN], f32).ap()
    ysh_d = nc.dram_tensor("ysh_d", [D, TN], f32).ap()
    h2sh_d = nc.dram_tensor("h2sh_d", [D, TN], bf16).ap()
    h2cat_d = nc.dram_tensor("h2cat_d", [NT, D, TN], bf16, addr_space="Shared").ap()
    mpart_d = nc.dram_tensor("mpart_d", [NT, D, TN], f32).ap()
    msh_d = nc.dram_tensor("msh_d", [D, TN], f32).ap()

    x_r = x_sh.rearrange("(ds p) t -> p ds t", p=P)
    hsh_r = hsh_d.rearrange("(ds p) t -> p ds t", p=P)
    hcat_r = hcat_d.rearrange("n (ds p) t -> p n ds t", p=P)
    ypart_r = ypart_d.rearrange("n (ds p) t -> p n ds t", p=P)
    ysh_r = ysh_d.rearrange("(ds p) t -> p ds t", p=P)
    h2sh_r = h2sh_d.rearrange("(ds p) t -> p ds t", p=P)
    h2cat_r = h2cat_d.rearrange("n (ds p) t -> p n ds t", p=P)
    mpart_r = mpart_d.rearrange("n (ds p) t -> p n ds t", p=P)
    msh_r = msh_d.rearrange("(ds p) t -> p ds t", p=P)

    with tile.TileContext(nc) as tc, ExitStack() as ctx:
        const_pool = ctx.enter_context(tc.tile_pool(name="const", bufs=1))
        big_pool = ctx.enter_context(tc.tile_pool(name="big", bufs=1))     # 32KB
        h_pool = ctx.enter_context(tc.tile_pool(name="h", bufs=2))         # 32KB
        q_pool = ctx.enter_context(tc.tile_pool(name="q", bufs=1))         # 8KB
        kv_pool = ctx.enter_context(tc.tile_pool(name="kv", bufs=1))       # 8KB
        att_pool = ctx.enter_context(tc.tile_pool(name="att", bufs=1))     # 8KB
        w_pool = ctx.enter_context(tc.tile_pool(name="w", bufs=3))         # 12KB
        tab_pool = ctx.enter_context(tc.tile_pool(name="tab", bufs=1))     # 10KB
        stage_pool = ctx.enter_context(tc.tile_pool(name="stage", bufs=3)) # 6KB
        sq_pool = ctx.enter_context(tc.tile_pool(name="sq", bufs=2))       # 4KB
        small_pool = ctx.enter_context(tc.tile_pool(name="small", bufs=3)) # 6KB
        ex_pool = ctx.enter_context(tc.tile_pool(name="ex", bufs=2))       # 2KB
        rope_pool = ctx.enter_context(tc.tile_pool(name="rope", bufs=3))   # 3KB
        rows_pool = ctx.enter_context(tc.tile_pool(name="rows", bufs=2))   # 8KB
        w8_pool = ctx.enter_context(tc.tile_pool(name="w8", bufs=3))       # 6KB
        psum = ctx.enter_context(tc.tile_pool(name="ps", bufs=2, space="PSUM"))

        ones_f = const_pool.tile([P, P], f32, tag="onesf")
        nc.vector.memset(ones_f, 1.0)
        ones_b = const_pool.tile([P, P], bf16, tag="onesb")
        nc.vector.memset(ones_b, 1.0)
        ident_f = const_pool.tile([P, P], f32, tag="identf")
        make_identity(nc, ident_f)
        ident_b = const_pool.tile([P, P], bf16, tag="identb")
        make_identity(nc, ident_b)
        eps_t = const_pool.tile([P, 1], f32, tag="eps")
        nc.vector.memset(eps_t, EPS)
        mlps_t = const_pool.tile([P, 2 * FC + DS], f32, tag="mlps")
        nc.sync.dma_start(mlps_t, mlps)
        atts_t = const_pool.tile([P, HC + DS], f32, tag="atts")
        nc.sync.dma_start(atts_t, atts)

        # cos/sin ship as bf16 and are upcast once; rope multiplies them with
        # f32 PSUM operands, which requires matching f32 dtype on the DVE.
        cosb = w_pool.tile([P, S], bf16, tag="w")
        nc.sync.dma_start(cosb[0:64, :], cosk)
        nc.sync.dma_start(cosb[64:128, :], cosk)
        cos_t = tab_pool.tile([P, S], f32, tag="cos")
        nc.vector.tensor_copy(out=cos_t, in_=cosb)
        sinb = w_pool.tile([P, S], bf16, tag="w")
        nc.sync.dma_start(sinb[0:64, :], sink)
        nc.sync.dma_start(sinb[64:128, :], sink)
        sin_t = tab_pool.tile([P, S], f32, tag="sin")
        nc.scalar.activation(sin_t[0:64, :], sinb[0:64, :], AF.Copy, scale=-1.0)
        nc.scalar.activation(sin_t[64:128, :], sinb[64:128, :], AF.Copy, scale=1.0)
        # Causal mask for the 4 diagonal key subtiles of a 512-query chunk:
        # mask[p, ks, j] = 1 if ks*128 + p <= j else 0.
        mask_t = tab_pool.tile([P, 4, TN], bf16, tag="mask")
        nc.gpsimd.memset(mask_t, 1.0)
        for ksl in range(4):
            nc.gpsimd.affine_select(
                out=mask_t[:, ksl, :], in_=mask_t[:, ksl, :],
                pattern=[[1, TN]], compare_op=OP.is_ge,
                fill=0.0, base=-128 * ksl, channel_multiplier=-1,
            )

        def rmsnorm_cast(xt, dst_bf):
            """dst_bf[:, i, :] (bf16) = rms-normalized xt[:, i, :] (f32)."""
            ps_ss = psum.tile([P, TN], f32, tag="proj")
            for i in range(DS):
                sq = sq_pool.tile([P, TN], f32, tag="sq")
                nc.vector.tensor_tensor(sq, xt[:, i, :], xt[:, i, :], OP.mult)
                nc.tensor.matmul(
                    ps_ss, lhsT=ones_f, rhs=sq, start=(i == 0), stop=(i == DS - 1)
                )
            sqv = small_pool.tile([P, TN], f32, tag="small")
            nc.scalar.activation(sqv, ps_ss, AF.Sqrt, bias=eps_t, scale=1.0 / D)
            rstd = small_pool.tile([P, TN], f32, tag="small")
            nc.vector.reciprocal(rstd, sqv)
            for i in range(DS):
                nc.vector.tensor_tensor(dst_bf[:, i, :], xt[:, i, :], rstd, OP.mult)

        def rope_bf(ps_in, pos, dst):
            """dst (bf16) = ps_in*cos + rotate_half(ps_in)*sin  (sin pre-signed).

            ps_in is the f32 PSUM projection; PSUM+SBUF operand mixes are
            exempt from the equal-base-partition rule, so the rotate-half
            cross-partition reads go straight from PSUM."""
            c_sl = cos_t[:, pos : pos + TN]
            s_sl = sin_t[:, pos : pos + TN]
            a = rope_pool.tile([P, TN], bf16, tag="rope")
            nc.vector.tensor_tensor(a, ps_in, c_sl, OP.mult)
            b = rope_pool.tile([P, TN], bf16, tag="rope")
            nc.vector.tensor_tensor(b[0:64, :], ps_in[64:128, :], s_sl[0:64, :], OP.mult)
            nc.vector.tensor_tensor(b[64:128, :], ps_in[0:64, :], s_sl[64:128, :], OP.mult)
            nc.vector.tensor_tensor(dst, a, b, OP.add)

        # ---------- Phase 0: rmsnorm own shard, AllGather ---------------------
        # x ships bf16 (the host re-adds the f32 x on the output path, so the
        # device copy only feeds the two rmsnorms); upcast once so downstream
        # DVE ops keep same-dtype operands.
        xb = h_pool.tile([P, DS, TN], bf16, tag="h")
        nc.sync.dma_start(xb, x_r)
        xt = big_pool.tile([P, DS, TN], f32, tag="big")
        nc.vector.tensor_copy(out=xt, in_=xb)
        hb = h_pool.tile([P, DS, TN], bf16, tag="h")
        rmsnorm_cast(xt, hb)
        nc.sync.dma_start(hsh_r, hb)
        nc.gpsimd.collective_compute(
            "AllGather", mybir.AluOpType.bypass, replica_groups=GRP,
            ins=[hsh_d], outs=[hcat_d],
        )

        # ---------- Phases 1-3 per batch -------------------------------------
        for b in range(B):
            # Phase 1: QKV projections + RoPE for this batch's 2048 tokens.
            qT = q_pool.tile([P, HC, S], bf16, tag="qT")
            kT = kv_pool.tile([P, S], bf16, tag="kT")
            vT = kv_pool.tile([P, S // P, P], bf16, tag="vT")
            for tcl in range(4):
                tcg = 4 * b + tcl
                pos = tcl * TN
                hc_t = h_pool.tile([P, DS, TN], bf16, tag="h")
                nc.sync.dma_start(hc_t, hcat_r[:, tcg, :, :])

                for h in range(HC):
                    wqt8 = w8_pool.tile([P, DS, P], i8, tag="w8")
                    nc.sync.dma_start(wqt8, wq[h])
                    wqt = w_pool.tile([P, DS, P], bf16, tag="w")
                    nc.vector.tensor_copy(out=wqt, in_=wqt8)
                    ps_q = psum.tile([P, TN], f32, tag="proj")
                    for i in range(DS):
                        nc.tensor.matmul(
                            ps_q, lhsT=wqt[:, i, :], rhs=hc_t[:, i, :],
                            start=(i == 0), stop=(i == DS - 1),
                        )
                    rope_bf(ps_q, pos, qT[:, h, pos : pos + TN])

                wkt8 = w8_pool.tile([P, DS, P], i8, tag="w8")
                nc.sync.dma_start(wkt8, wk)
                wkt = w_pool.tile([P, DS, P], bf16, tag="w")
                nc.vector.tensor_copy(out=wkt, in_=wkt8)
                ps_k = psum.tile([P, TN], f32, tag="proj")
                for i in range(DS):
                    nc.tensor.matmul(
                        ps_k, lhsT=wkt[:, i, :], rhs=hc_t[:, i, :],
                        start=(i == 0), stop=(i == DS - 1),
                    )
                rope_bf(ps_k, pos, kT[:, pos : pos + TN])

                wvt8 = w8_pool.tile([P, DS, P], i8, tag="w8")
                nc.sync.dma_start(wvt8, wv)
                wvt = w_pool.tile([P, DS, P], bf16, tag="w")
                nc.vector.tensor_copy(out=wvt, in_=wvt8)
                ps_v = psum.tile([P, TN], f32, tag="proj")
                for i in range(DS):
                    nc.tensor.matmul(
                        ps_v, lhsT=wvt[:, i, :], rhs=hc_t[:, i, :],
                        start=(i == 0), stop=(i == DS - 1),
                    )
                vts = stage_pool.tile([P, TN], bf16, tag="stb")
                nc.scalar.copy(vts, ps_v)
                for t in range(4):
                    ps_tr = psum.tile([P, P], bf16, tag="den")
                    nc.tensor.transpose(ps_tr, vts[:, t * P : (t + 1) * P], ident_b)
                    nc.vector.tensor_copy(out=vT[:, tcl * 4 + t, :], in_=ps_tr)

            # Phase 2: attention for this batch (2 heads x 4 query chunks).
            attT = att_pool.tile([P, HC, S], bf16, tag="attT")
            for h in range(HC):
                for qcl in range(4):
                    q0 = qcl * TN
                    nks = (qcl + 1) * 4          # visible key subtiles
                    ps_att = psum.tile([P, TN], f32, tag="att")
                    ps_den = psum.tile([P, TN], f32, tag="score")
                    for ks in range(nks):
                        ps_s = psum.tile([P, TN], f32, tag="proj")
                        nc.tensor.matmul(
                            ps_s, lhsT=kT[:, ks * P : (ks + 1) * P],
                            rhs=qT[:, h, q0 : q0 + TN],
                            start=True, stop=True,
                        )
                        ex = ex_pool.tile([P, TN], bf16, tag="ex")
                        nc.scalar.activation(ex, ps_s, AF.Exp, scale=atts_t[:, h : h + 1])
                        if ks >= nks - 4:
                            nc.vector.tensor_tensor(
                                ex, ex, mask_t[:, ks - (nks - 4), :], OP.mult
                            )
                        nc.tensor.matmul(
                            ps_att, lhsT=vT[:, ks, :], rhs=ex,
                            start=(ks == 0), stop=(ks == nks - 1),
                        )
                        nc.tensor.matmul(
                            ps_den, lhsT=ones_b, rhs=ex,
                            start=(ks == 0), stop=(ks == nks - 1),
                        )
                    rec = small_pool.tile([P, TN], f32, tag="small")
                    nc.vector.reciprocal(rec, ps_den)
                    nc.vector.tensor_tensor(
                        attT[:, h, q0 : q0 + TN], ps_att, rec, OP.mult
                    )

            # Phase 3: o-proj partial sums for this batch's 4 token chunks.
            for mc in range(DS):
                wot8 = w8_pool.tile([P, HC, P], i8, tag="w8")
                nc.sync.dma_start(wot8, wo[mc])
                wot = w_pool.tile([P, HC, P], bf16, tag="w")
                nc.vector.tensor_copy(out=wot, in_=wot8)
                for tcl in range(4):
                    tcg = 4 * b + tcl
                    ps_o = psum.tile([P, TN], f32, tag="att")
                    for h in range(HC):
                        nc.tensor.matmul(
                            ps_o, lhsT=wot[:, h, :],
                            rhs=attT[:, h, tcl * TN : (tcl + 1) * TN],
                            start=(h == 0), stop=(h == HC - 1),
                        )
                    st = stage_pool.tile([P, TN], f32, tag="stf")
                    nc.scalar.activation(
                        st, ps_o, AF.Copy, scale=atts_t[:, HC + mc : HC + mc + 1]
                    )
                    nc.sync.dma_start(ypart_r[:, tcg, mc, :], st)

        nc.gpsimd.collective_compute(
            "ReduceScatter", mybir.AluOpType.add, replica_groups=GRP,
            ins=[ypart_d], outs=[ysh_d],
        )

        # ---------- Phase 4: residual + rmsnorm2 + AllGather ------------------
        yt = big_pool.tile([P, DS, TN], f32, tag="big")
        for i in range(DS):
            ys = stage_pool.tile([P, TN], f32, tag="stf")
            nc.sync.dma_start(ys, ysh_r[:, i, :])
            xb2 = stage_pool.tile([P, TN], bf16, tag="stb")
            nc.sync.dma_start(xb2, x_r[:, i, :])
            xs = stage_pool.tile([P, TN], f32, tag="stf")
            nc.vector.tensor_copy(out=xs, in_=xb2)
            nc.vector.tensor_tensor(yt[:, i, :], ys, xs, OP.add)
        h2b = h_pool.tile([P, DS, TN], bf16, tag="h")
        rmsnorm_cast(yt, h2b)
        nc.sync.dma_start(h2sh_r, h2b)
        nc.gpsimd.collective_compute(
            "AllGather", mybir.AluOpType.bypass, replica_groups=GRP,
            ins=[h2sh_d], outs=[h2cat_d],
        )

        # ---------- Phase 5: SwiGLU MLP (own FF shard, all tokens) -----------
        for tcg in range(NT):
            hc_t = h_pool.tile([P, DS, TN], bf16, tag="h")
            nc.sync.dma_start(hc_t, h2cat_r[:, tcg, :, :])
            hid = att_pool.tile([P, FC, TN], bf16, tag="hid")
            for j in range(FC):
                wgt8 = w8_pool.tile([P, DS, P], i8, tag="w8")
                nc.sync.dma_start(wgt8, wg[j])
                wgt = w_pool.tile([P, DS, P], bf16, tag="w")
                nc.vector.tensor_copy(out=wgt, in_=wgt8)
                ps_g = psum.tile([P, TN], f32, tag="score")
                for i in range(DS):
                    nc.tensor.matmul(
                        ps_g, lhsT=wgt[:, i, :], rhs=hc_t[:, i, :],
                        start=(i == 0), stop=(i == DS - 1),
                    )
                sg = stage_pool.tile([P, TN], bf16, tag="stb")
                nc.scalar.activation(sg, ps_g, AF.Silu, scale=mlps_t[:, j : j + 1])
                wut8 = w8_pool.tile([P, DS, P], i8, tag="w8")
                nc.sync.dma_start(wut8, wu[j])
                wut = w_pool.tile([P, DS, P], bf16, tag="w")
                nc.vector.tensor_copy(out=wut, in_=wut8)
                ps_u = psum.tile([P, TN], f32, tag="att")
                for i in range(DS):
                    nc.tensor.matmul(
                        ps_u, lhsT=wut[:, i, :], rhs=hc_t[:, i, :],
                        start=(i == 0), stop=(i == DS - 1),
                    )
                us = stage_pool.tile([P, TN], bf16, tag="stb")
                nc.scalar.activation(
                    us, ps_u, AF.Copy, scale=mlps_t[:, FC + j : FC + j + 1]
                )
                nc.vector.tensor_tensor(hid[:, j, :], us, sg, OP.mult)
            for mc in range(DS):
                wdt8 = w8_pool.tile([P, FC, P], i8, tag="w8")
                nc.sync.dma_start(wdt8, wd[mc])
                wdt = w_pool.tile([P, FC, P], bf16, tag="w")
                nc.vector.tensor_copy(out=wdt, in_=wdt8)
                ps_d = psum.tile([P, TN], f32, tag="proj")
                for j in range(FC):
                    nc.tensor.matmul(
                        ps_d, lhsT=wdt[:, j, :], rhs=hid[:, j, :],
                        start=(j == 0), stop=(j == FC - 1),
                    )
                st = stage_pool.tile([P, TN], f32, tag="stf")
                nc.scalar.activation(
                    st, ps_d, AF.Copy, scale=mlps_t[:, 2 * FC + mc : 2 * FC + mc + 1]
                )
                nc.sync.dma_start(mpart_r[:, tcg, mc, :], st)

        nc.gpsimd.collective_compute(
            "ReduceScatter", mybir.AluOpType.add, replica_groups=GRP,
            ins=[mpart_d], outs=[msh_d],
        )

        # ---------- Phase 6: contribution = attn + mlp, transpose, store ------
        ctt = h_pool.tile([P, DS, TN], bf16, tag="h")
        for mc in range(DS):
            ys = stage_pool.tile([P, TN], f32, tag="stf")
            nc.sync.dma_start(ys, ysh_r[:, mc, :])
            ms = stage_pool.tile([P, TN], f32, tag="stf")
            nc.sync.dma_start(ms, msh_r[:, mc, :])
            nc.vector.tensor_tensor(ctt[:, mc, :], ys, ms, OP.add)
        for qs in range(TN // P):
            rows = rows_pool.tile([P, DS, P], bf16, tag="rows")
            for mc in range(DS):
                ps_tr = psum.tile([P, P], bf16, tag="den")
                nc.tensor.transpose(ps_tr, ctt[:, mc, qs * P : (qs + 1) * P], ident_b)
                nc.vector.tensor_copy(out=rows[:, mc, :], in_=ps_tr)
            nc.sync.dma_start(out_rows[qs * P : (qs + 1) * P, :], rows)

    nc.compile()
    return nc


def _pack_lhsT(w):
    """[M, K] row-major -> lhsT tile layout:
    out[mc, p, ks, c] = w[mc*128 + c, ks*128 + p]."""
    M, K = w.shape
    w4 = w.reshape(M // P, P, K // P, P)  # [mc, c, ks, p]
    return np.ascontiguousarray(w4.transpose(0, 3, 2, 1))


def _prep_inputs(inputs):
    bf = ml_dtypes.bfloat16
    x = np.asarray(inputs["x"], np.float32)
    cos = np.asarray(inputs["cos"], np.float32)
    sin = np.asarray(inputs["sin"], np.float32)
    g1 = np.asarray(inputs["g1"], np.float32)
    g2 = np.asarray(inputs["g2"], np.float32)

    scale = 1.0 / np.sqrt(np.float32(HD))
    wq_f = np.asarray(inputs["wq"], np.float32) * g1[None, :] * scale
    wk_f = np.asarray(inputs["wk"], np.float32) * g1[None, :]
    wv_f = np.asarray(inputs["wv"], np.float32) * g1[None, :]
    wo_f = np.asarray(inputs["wo"], np.float32)
    wg_f = np.asarray(inputs["w_gate"], np.float32) * g2[None, :]
    wu_f = np.asarray(inputs["w_up"], np.float32) * g2[None, :]
    wd_f = np.asarray(inputs["w_down"], np.float32)

    wg_pad = np.zeros((NCORES * FFP, D), np.float32)
    wg_pad[:FF] = wg_f
    wu_pad = np.zeros((NCORES * FFP, D), np.float32)
    wu_pad[:FF] = wu_f
    wd_pad = np.zeros((D, NCORES * FFP), np.float32)
    wd_pad[:, :FF] = wd_f

    cosT = np.ascontiguousarray(cos.T[: HD // 2]).astype(bf)  # [64, S] half table
    sinT = np.ascontiguousarray(sin.T[: HD // 2]).astype(bf)  # unsigned half

    xT_b = [np.ascontiguousarray(x[b].T) for b in range(B)]  # [D, S]

    in_maps = []
    for c in range(NCORES):
        b, qi = c // 4, c % 4
        wq_c = wq_f[2 * c * HD : (2 * c + HC) * HD]
        wk_c = wk_f[c * HD : (c + 1) * HD]
        wv_c = wv_f[c * HD : (c + 1) * HD]
        wo_c = wo_f[:, 2 * c * HD : (2 * c + HC) * HD]
        wg_c = wg_pad[c * FFP : (c + 1) * FFP]
        wu_c = wu_pad[c * FFP : (c + 1) * FFP]
        wd_c = wd_pad[:, c * FFP : (c + 1) * FFP]

        def quant8_rows(w):
            """Per-output-channel (row) absmax int8 quantization."""
            s = np.maximum(np.abs(w).max(axis=1), 1e-12) / 127.0   # [M]
            q = np.round(w / s[:, None]).astype(np.int8)
            return q, s.astype(np.float32)

        def quant8_block(w):
            """Whole-block scalar absmax int8 quantization."""
            s = max(float(np.abs(w).max()) / 127.0, 1e-12)
            return np.round(w / s).astype(np.int8), np.float32(s)

        wg_q, s_g = quant8_rows(wg_c)
        wu_q, s_u = quant8_rows(wu_c)
        wd_q, s_d = quant8_rows(wd_c)
        mlp_s = np.ascontiguousarray(
            np.concatenate(
                [s_g.reshape(FC, P).T, s_u.reshape(FC, P).T, s_d.reshape(DS, P).T],
                axis=1,
            )
        )

        # Attention: per-head scalar scales for q (rope-linear), k, v; per-row
        # for wo.  exp scale col h = s_q[h]*s_k; o-proj scale col = s_wo*s_v.
        wq_q = np.empty_like(wq_c, dtype=np.int8)
        s_qh = []
        for hh in range(HC):
            blk, s_b = quant8_block(wq_c[hh * HD : (hh + 1) * HD])
            wq_q[hh * HD : (hh + 1) * HD] = blk
            s_qh.append(s_b)
        wk_q, s_k = quant8_block(wk_c)
        wv_q, s_v = quant8_block(wv_c)
        wo_q, s_wo = quant8_rows(wo_c)
        att_s = np.ascontiguousarray(
            np.concatenate(
                [
                    np.broadcast_to(
                        np.array([s_qh[0] * s_k, s_qh[1] * s_k], np.float32), (P, HC)
                    ),
                    (s_wo * s_v).reshape(DS, P).T,
                ],
                axis=1,
            )
        )
        in_maps.append(
            dict(
                x_sh=np.ascontiguousarray(xT_b[b][:, qi * TN : (qi + 1) * TN]).astype(bf),
                wq_pk=_pack_lhsT(wq_q),
                wk_pk=_pack_lhsT(wk_q)[0],
                wv_pk=_pack_lhsT(wv_q)[0],
                wo_pk=_pack_lhsT(wo_q),
                att_s=att_s,
                wg_pk=_pack_lhsT(wg_q),
                wu_pk=_pack_lhsT(wu_q),
                wd_pk=_pack_lhsT(wd_q),
                mlp_s=mlp_s,
                cos_k=cosT,
                sin_k=sinT,
            )
        )
    return in_maps


def kernel(**inputs):
    global _prog
    from concourse.bass_utils import run_bass_kernel_spmd

    try:
        import jax

        inputs = jax.device_get(inputs)  # batch host pull if device-resident
    except Exception:
        pass
    if _prog is None:
        _prog = _build()
    in_maps = _prep_inputs(inputs)
    res = run_bass_kernel_spmd(_prog, in_maps, list(range(NCORES)))
    x = np.asarray(inputs["x"], np.float32)
    out = np.empty((B, S, D), np.float32)
    for c in range(NCORES):
        b, q0 = c // 4, (c % 4) * TN
        out[b, q0 : q0 + TN, :] = x[b, q0 : q0 + TN, :] + res.results[c][
            "out_rows"
        ].astype(np.float32)
    return out
